# revision 13
# baseline (speedup 1.0000x reference)
"""Trainium2 Bass kernel for nn_Attention_48206712930624.

Dense transformer block: LayerNorm -> QKV proj -> 8-head attention
(head_dim = 512) -> output projection.  B=4, S=2048, D=512, H=8.

Sharding: tensor-parallel over heads -- each of the 8 NeuronCores computes
one head end-to-end (LN duplicated), producing a partial output projection
Y_h = (P_h @ V_h) @ o_w_h (un-normalized) plus the softmax denominators
l_h.  The host combines:  out = sum_h Y_h / l_h + const.

Device-side design notes (v2):
  * All big matmuls run in bf16 with fp32 PSUM accumulation -- bf16 is
    full PE rate (1 cycle/row) like f32r at N=512, but transposes drop
    from 2.0 (f32) to 1.0 cycles/row, SBUF footprint halves, and DVE
    elementwise ops hit the 2x 16-bit mode.
  * Scores are computed TRANSPOSED (S^T[k,q] = k^T.T @ q^T) so softmax's
    exp is orientation-agnostic (ACT elementwise) and P^T lands directly
    in the layout att@V needs (k on partitions).  No max-subtraction
    (logits ~ N(0,1); folded scale keeps exp well within range).
  * Softmax denominators l[q] = sum_k P^T[k,q]: the 16 k-chunks are
    folded 16->1 with a DVE add tree (free-axis strided adds), then ONE
    [128->1] ones-matmul per q-block reduces the partition axis.  This
    replaces the 16-matmul PE chain of v1 (-129k PE cycles/core).
  * LayerNorm scale/bias and the attention scale are folded into the
    weights on the host; v-bias and o_b fold into a constant row added on
    the host.  rstd = exp(-0.5*ln(var+eps)) keeps the whole kernel on ONE
    ACT table set (natural_log_exp_and_others).
  * Engine split: PE matmuls/transposes; ACT exp + PSUM->SBUF copies
    (oT, yt, xhT); DVE stats/normalize/bias-adds/v-copy/l-tree.
  * Batch-level software pipeline: batch b+1's LayerNorm+transpose is
    emitted in 8 fine-grained slices sprinkled between attention chunks
    of batch b (one slice per hook), so the PE never stalls on the
    serial LN chain.  The b=3 slices compute batch 0 for the NEXT For_i
    iteration (wrap-around hoist into a dedicated bufs=1 slot), so the
    repeat loop pipelines across iterations as well.
"""

import sys

import numpy as np

for _p in ("/opt/trn_rl_repo", "/root/.axon_site/_ro/trn_rl_repo"):
    if _p not in sys.path:
        sys.path.append(_p)

import concourse.bacc as bacc
import concourse.mybir as mybir
import concourse.tile as tile
from concourse.bass_utils import run_bass_kernel_spmd
from concourse.masks import make_identity

# Steer the ACT-table-load placement pass to the one set that holds every
# function this kernel uses (ln, exp, copy, identity), so the whole kernel
# runs on a single table load instead of thrashing between per-function
# sets.  Only the pass's view is doctored; runtime tables are untouched.
_ONE_SET = "natural_log_exp_and_others"
_orig_get_act_tables = bacc.get_activation_tables


def _patched_get_act_tables(arch):
    t = _orig_get_act_tables(arch)
    af = mybir.ActivationFunctionType
    strip = {af.Ln, af.Exp, af.Copy, af.Identity}
    return {
        name: (set(fns) if name == _ONE_SET else set(fns) - strip)
        for name, fns in t.items()
    }


bacc.get_activation_tables = _patched_get_act_tables

B, S, D, H = 4, 2048, 512, 8
P = 128
DC = D // P          # head/model dim chunks (4)
KC = S // P          # k chunks per batch (16)
QB = 512             # q-block size
NQB = S // QB        # q blocks per batch (4)
EPS = 1e-5
F32 = mybir.dt.float32
F32R = mybir.dt.float32r
BF16 = mybir.dt.bfloat16
AF = mybir.ActivationFunctionType
ALU = mybir.AluOpType

N_CORES = 8

_CACHE = {}


class _Kern:
    """Holds the pools/constants and emits the per-batch phases."""

    def __init__(self, nc, tc, pools):
        self.nc = nc
        self.tc = tc
        for k, v in pools.items():
            setattr(self, k, v)

    def setup_consts(self, qb_d, kb_d, w_drams):
        nc = self.nc
        self.ident = self.const.tile([P, P], BF16, name="ident")
        make_identity(nc, self.ident)
        ones_raw = self.const.tile([P, 1], F32, name="ones_raw")
        nc.vector.memset(ones_raw, 1.0)
        self.ones_r = self.const.tile([P, 1], F32R, name="ones_r")
        nc.vector.tensor_copy(self.ones_r, ones_raw.bitcast(F32R))
        self.eps_t = self.const.tile([P, 1], F32, name="eps_t")
        nc.vector.memset(self.eps_t, EPS)
        self.qb_t = self.const.tile([P, DC], F32, name="qb_t")
        nc.gpsimd.dma_start(out=self.qb_t,
                            in_=qb_d.rearrange("(c p) -> p c", p=P))
        self.kb_t = self.const.tile([P, DC], F32, name="kb_t")
        nc.gpsimd.dma_start(out=self.kb_t,
                            in_=kb_d.rearrange("(c p) -> p c", p=P))
        # weights: load f32 (SWDGE queue, so x loads aren't stuck behind)
        # into the pT-slot staging area, then cast to bf16
        self.w_r = {}
        for n, dram in w_drams.items():
            wst = self.ptp.tile([P, DC, D], F32, name=f"{n}_stage", tag="pT")
            nc.gpsimd.dma_start(out=wst,
                                in_=dram.rearrange("(c p) n -> p c n", p=P))
            self.w_r[n] = self.wts.tile([P, DC, D], BF16, name=f"{n}_r", tag=n)
            nc.scalar.copy(self.w_r[n], wst)

    # ---- phase A: LayerNorm + transpose -> xhT [d, r], split slices ----
    def phase_a_steps(self, x, b):
        """Allocate xhT(b) and return (xhT, stats_steps[8], transp_steps[8]).

        stats_steps[g]: DMA load + bn stats + rstd + bf16 normalize into
        the persistent xh_all tile.  Emit these EARLY at ACT-quiet spots
        (rstd must not queue behind a fresh block of pending exps).
        transp_steps[g]: 8 PE transposes + 2 ACT copies into xhT.  Emit
        these LATE at hook points -- by then the stats are long done, so
        the in-order PE queue never waits on the LN chain."""
        nc = self.nc
        pool = self.xh0 if b == 0 else self.xht
        xhT = pool.tile([P, DC, S], BF16, name=f"xhT{b}", tag="xht")
        xha = self.xhap.tile([P, KC, D], BF16, name=f"xha{b}", tag="xha")

        def mk_stats(g):
            def emit():
                xg = self.stage.tile([P, 2, D], F32, name="xg", tag="xg",
                                     bufs=2)
                r0 = g * 2 * P
                nc.sync.dma_start(
                    out=xg,
                    in_=x[b, r0:r0 + 2 * P, :].rearrange(
                        "(j p) d -> p j d", p=P))
                mvs, rstds = [], []
                for j in range(2):
                    st6 = self.stats.tile([P, 6], F32, name="st6",
                                          tag=f"st6{j}")
                    nc.vector.bn_stats(out=st6, in_=xg[:, j, :])
                    mv = self.stats.tile([P, 2], F32, name="mv", tag=f"mv{j}")
                    nc.vector.bn_aggr(out=mv, in_=st6)
                    mvs.append(mv)
                for j in range(2):
                    # rstd = exp(-0.5 * ln(var + eps))
                    lnv = self.stats.tile([P, 1], F32, name="lnv",
                                          tag=f"lnv{j}")
                    nc.scalar.activation(out=lnv, in_=mvs[j][:, 1:2],
                                         func=AF.Ln, bias=self.eps_t)
                    rstd = self.stats.tile([P, 1], F32, name="rstd",
                                           tag=f"rstd{j}")
                    nc.scalar.activation(out=rstd, in_=lnv, func=AF.Exp,
                                         scale=-0.5)
                    rstds.append(rstd)
                for j in range(2):
                    rt = g * 2 + j
                    nc.vector.tensor_scalar(out=xha[:, rt, :],
                                            in0=xg[:, j, :],
                                            scalar1=mvs[j][:, 0:1],
                                            scalar2=rstds[j],
                                            op0=ALU.subtract, op1=ALU.mult)
            return emit

        def mk_transp(g):
            def emit():
                for j in range(2):
                    rt = g * 2 + j
                    tp = self.psum.tile([P, D], BF16, name="tp", tag="t",
                                        bufs=2)
                    for dc in range(DC):
                        nc.tensor.transpose(tp[:, dc * P:(dc + 1) * P],
                                            xha[:, rt, dc * P:(dc + 1) * P],
                                            self.ident)
                    nc.scalar.copy(
                        out=xhT[:, :, rt * P:(rt + 1) * P],
                        in_=tp.rearrange("p (c r) -> p c r", c=DC))
            return emit

        n = KC // 2
        return xhT, [mk_stats(g) for g in range(n)], \
            [mk_transp(g) for g in range(n)]

    # ---- phase B: kT in two column-halves, v as 8 row-group closures ----
    # scores group kp reads kT[:, all dc, kc-pair] -- the hf=0 kT groups
    # unlock sps groups 0..3, so kT is emitted hf-outer and the second
    # half interleaves with the first scores groups.
    def phase_b_k_half(self, xhT, kT, hf):
        nc = self.nc
        outs = []
        for cc in range(DC):
            def emit(cc=cc, hf=hf):
                kps = self.psum.tile([P, 2, QB], F32, name="kps", tag="s",
                                     bufs=2)
                for dc in range(DC):
                    for j in range(2):
                        q0 = (hf * 2 + j) * QB
                        nc.tensor.matmul(
                            kps[:, j, :],
                            self.w_r["kw"][:, dc, cc * P:(cc + 1) * P],
                            xhT[:, dc, q0:q0 + QB],
                            start=(dc == 0), stop=(dc == DC - 1))
                nc.vector.tensor_scalar_add(
                    out=kT[:, cc, hf * 2 * QB:(hf + 1) * 2 * QB],
                    in0=kps.rearrange("p j q -> p (j q)"),
                    scalar1=self.kb_t[:, cc:cc + 1])
            outs.append(emit)
        return outs

    def phase_b_v_groups(self, xhT, b):
        nc = self.nc
        v_t = self.kv.tile([P, KC, D], BF16, name=f"v{b}", tag="v")

        def mk(rp):
            def emit():
                vps = self.psum.tile([P, 2, D], F32, name="vps", tag="s",
                                     bufs=2)
                for dc in range(DC):
                    for j in range(2):
                        rc = rp * 2 + j
                        nc.tensor.matmul(
                            vps[:, j, :], xhT[:, dc, rc * P:(rc + 1) * P],
                            self.w_r["vw"][:, dc, :],
                            start=(dc == 0), stop=(dc == DC - 1))
                nc.vector.tensor_copy(out=v_t[:, rp * 2:rp * 2 + 2, :],
                                      in_=vps)
            return emit

        return v_t, [mk(rp) for rp in range(KC // 2)]

    # ---- q^T projection for one q-block ----
    def qproj(self, xhT, qb_i):
        nc = self.nc
        q0 = qb_i * QB
        qT = self.qtp.tile([P, DC, QB], BF16, name=f"qT{qb_i}", tag="qT")
        for cp in range(DC // 2):
            qps = self.psum.tile([P, 2, QB], F32, name="qps", tag="s", bufs=2)
            for dc in range(DC):
                for j in range(2):
                    cc = cp * 2 + j
                    nc.tensor.matmul(
                        qps[:, j, :],
                        self.w_r["qw"][:, dc, cc * P:(cc + 1) * P],
                        xhT[:, dc, q0:q0 + QB],
                        start=(dc == 0), stop=(dc == DC - 1))
            for j in range(2):
                cc = cp * 2 + j
                nc.vector.tensor_scalar_add(out=qT[:, cc, :],
                                            in0=qps[:, j, :],
                                            scalar1=self.qb_t[:, cc:cc + 1])
        return qT

    # ---- attention scores: S^T + exp, as 8 group closures ----
    def scores_groups(self, qT, kT):
        nc = self.nc
        pT = self.ptp.tile([P, KC, QB], BF16, name="pT", tag="pT")

        def mk(kp):
            def emit():
                sps = self.psum.tile([P, 2, QB], F32, name="sps", tag="s",
                                     bufs=2)
                for dc in range(DC):
                    for j in range(2):
                        kc = kp * 2 + j
                        nc.tensor.matmul(
                            sps[:, j, :], kT[:, dc, kc * P:(kc + 1) * P],
                            qT[:, dc, :],
                            start=(dc == 0), stop=(dc == DC - 1))
                nc.scalar.activation(out=pT[:, kp * 2:kp * 2 + 2, :],
                                     in_=sps, func=AF.Exp)
            return emit

        return pT, [mk(kp) for kp in range(KC // 2)]

    # ---- softmax denominator, DVE part: tree-fold the 16 k-chunks ----
    def attn_l_tree(self, pT):
        nc = self.nc
        t8 = self.tree.tile([P, 8, QB], BF16, name="t8", tag="t8", bufs=1)
        nc.vector.tensor_tensor(out=t8, in0=pT[:, 0:8, :],
                                in1=pT[:, 8:16, :], op=ALU.add)
        nc.vector.tensor_tensor(out=t8[:, 0:4, :], in0=t8[:, 0:4, :],
                                in1=t8[:, 4:8, :], op=ALU.add)
        nc.vector.tensor_tensor(out=t8[:, 0:2, :], in0=t8[:, 0:2, :],
                                in1=t8[:, 2:4, :], op=ALU.add)
        t1 = self.tree.tile([P, QB], F32R, name="t1", tag="t1", bufs=2)
        nc.vector.tensor_tensor(out=t1, in0=t8[:, 0, :], in1=t8[:, 1, :],
                                op=ALU.add)
        return t1

    # ---- softmax denominator, PE part: one [128->1] ones-matmul ----
    # (emitted AFTER att@V so the PE never waits on the exp+tree chain)
    def attn_l_fin(self, lsum, t1, b, qb_i):
        nc = self.nc
        q0 = qb_i * QB
        l_ps = self.psum.tile([1, QB], F32, name="l_ps", tag="t", bufs=2)
        nc.tensor.matmul(l_ps, self.ones_r, t1,
                         start=True, stop=True)
        l_sb = self.lsbp.tile([1, QB], F32, name="l_sb", tag="l", bufs=2)
        nc.scalar.copy(out=l_sb, in_=l_ps)
        nc.sync.dma_start(out=lsum[b, q0:q0 + QB].unsqueeze(0), in_=l_sb)

    # ---- attention att@V, as 4 per-dc closures ----
    def av_dcs(self, pT, v_t):
        nc = self.nc
        oT = self.otp.tile([P, DC, QB], BF16, name="oT", tag="oT")

        def mk(dc):
            def emit():
                o_ps = self.psum.tile([P, QB], F32, name="o_ps", tag="o",
                                      bufs=2)
                for kc in range(KC):
                    nc.tensor.matmul(o_ps, v_t[:, kc, dc * P:(dc + 1) * P],
                                     pT[:, kc, :],
                                     start=(kc == 0), stop=(kc == KC - 1))
                nc.scalar.copy(out=oT[:, dc, :], in_=o_ps)
            return emit

        return oT, [mk(dc) for dc in range(DC)]

    # ---- attention tail: output projection + store ----
    def attn_tail(self, y, oT, b, qb_i):
        nc = self.nc
        q0 = qb_i * QB
        for qc in range(QB // P):
            yps = self.psum.tile([P, D], F32, name="yps", tag="o", bufs=2)
            for dc in range(DC):
                nc.tensor.matmul(yps, oT[:, dc, qc * P:(qc + 1) * P],
                                 self.w_r["ow"][:, dc, :],
                                 start=(dc == 0), stop=(dc == DC - 1))
            yt = self.stage.tile([P, D], F32, name="yt", tag="yt", bufs=3)
            nc.scalar.copy(out=yt, in_=yps)
            r0 = q0 + qc * P
            nc.sync.dma_start(out=y[b, r0:r0 + P, :], in_=yt)


def build(repeat=None, phases="full"):
    """repeat=R wraps the whole compute in a hardware For_i loop that runs
    it R times -- used only for wall-clock device-time benchmarking.
    phases in {"A", "AB", "ABS", "full"} truncates the pipeline (bench)."""
    import contextlib

    nc = bacc.Bacc("TRN2", target_bir_lowering=False, debug=False,
                   num_devices=N_CORES)
    x = nc.dram_tensor("x", [B, S, D], F32, kind="ExternalInput").ap()
    w_drams = {
        n: nc.dram_tensor(n, [D, D], F32, kind="ExternalInput").ap()
        for n in ("qw", "kw", "vw", "ow")
    }
    qb_d = nc.dram_tensor("qb", [D], F32, kind="ExternalInput").ap()
    kb_d = nc.dram_tensor("kb", [D], F32, kind="ExternalInput").ap()
    y = nc.dram_tensor("y", [B, S, D], F32, kind="ExternalOutput").ap()
    lsum = nc.dram_tensor("lsum", [B, S], F32, kind="ExternalOutput").ap()

    with tile.TileContext(nc) as tc:
        with (
            tc.tile_pool(name="const", bufs=1) as const,
            tc.tile_pool(name="wts", bufs=1) as wts,
            tc.tile_pool(name="kv", bufs=1) as kv,
            tc.tile_pool(name="xh0", bufs=1) as xh0,
            tc.tile_pool(name="xht", bufs=2) as xht,
            tc.tile_pool(name="xha", bufs=1) as xhap,
            tc.tile_pool(name="ptp", bufs=2) as ptp,
            tc.tile_pool(name="qt", bufs=2) as qtp,
            tc.tile_pool(name="ot", bufs=2) as otp,
            tc.tile_pool(name="stage", bufs=1) as stage,
            tc.tile_pool(name="stats", bufs=4) as stats,
            tc.tile_pool(name="tree", bufs=1) as tree,
            tc.tile_pool(name="lsb", bufs=1) as lsbp,
            tc.tile_pool(name="psum", bufs=1, space="PSUM") as psum,
        ):
            k = _Kern(nc, tc, dict(const=const, wts=wts, kv=kv, xh0=xh0,
                                   xht=xht, xhap=xhap, ptp=ptp, qtp=qtp,
                                   otp=otp, stage=stage, stats=stats,
                                   tree=tree, lsbp=lsbp, psum=psum))
            k.setup_consts(qb_d, kb_d, w_drams)

            loop_cm = (tc.For_i(0, repeat, 1) if repeat
                       else contextlib.nullcontext())
            with loop_cm:
                # batch 0's LN at body top (loop-carried RAW into the next
                # For_i iteration deadlocks Tile's semaphore scheme); its
                # transposes interleave with batch 0's v-groups below
                xhT, sts, trs = k.phase_a_steps(x, 0)
                for st in sts:
                    st()
                for b in range(B):
                    if phases in ("A", "AB"):
                        for tr in trs:
                            tr()
                        if phases == "AB":
                            kT = k.kv.tile([P, DC, S], BF16, name=f"kT{b}",
                                           tag="kT")
                            v_t, vgs = k.phase_b_v_groups(xhT, b)
                            for vg in vgs:
                                vg()
                            for e in k.phase_b_k_half(xhT, kT, 0):
                                e()
                            for e in k.phase_b_k_half(xhT, kT, 1):
                                e()
                        if b + 1 < B:
                            xhT, sts, trs = k.phase_a_steps(x, b + 1)
                            for st in sts:
                                st()
                        continue
                    skip_av = phases == "ABS"

                    # -- projections: v-groups (batch 0: interleaved with
                    # its own transposes), kT half 0, qproj(0) --
                    kT = k.kv.tile([P, DC, S], BF16, name=f"kT{b}", tag="kT")
                    v_t, vgs = k.phase_b_v_groups(xhT, b)
                    for g in range(KC // 2):
                        if b == 0 and trs:
                            trs.pop(0)()
                        vgs[g]()
                    for e in k.phase_b_k_half(xhT, kT, 0):
                        e()
                    qT = {0: k.qproj(xhT, 0)}

                    # -- scores(0) groups 0..3 interleaved with kT half 1;
                    # groups 4..7 after qproj(1) --
                    pT = {}
                    t1 = {}
                    pT[0], sgs = k.scores_groups(qT[0], kT)
                    kh1 = k.phase_b_k_half(xhT, kT, 1)
                    for g in range(4):
                        sgs[g]()
                        kh1[g]()
                    qT[1] = k.qproj(xhT, 1)
                    for g in range(4, 8):
                        sgs[g]()
                    t1[0] = k.attn_l_tree(pT[0])

                    # hoisted LN for b+1: stats early, transposes one
                    # block later (their deps are then long since ready)
                    if b + 1 < B:
                        nxt, sts, trs = k.phase_a_steps(x, b + 1)
                    else:
                        nxt, sts, trs = None, [], []

                    def spot2(lst):
                        if lst:
                            lst.pop(0)()
                        if lst:
                            lst.pop(0)()

                    spot2(sts)
                    # -- 4 blocks: scores(i+1) groups laced 2:1 with
                    # av(i) dc-streams, then l_fin/qproj/tail --
                    for i in range(NQB):
                        if i + 1 < NQB:
                            pT[i + 1], sg2 = k.scores_groups(qT[i + 1], kT)
                        else:
                            sg2 = []
                        if not skip_av:
                            oT, avs = k.av_dcs(pT[i], v_t)
                        else:
                            avs = [lambda: None] * DC
                        si = 0
                        for a in range(DC):
                            for _ in range(2):
                                if si < len(sg2):
                                    sg2[si]()
                                    si += 1
                            avs[a]()
                        while si < len(sg2):
                            sg2[si]()
                            si += 1
                        if i + 1 < NQB:
                            t1[i + 1] = k.attn_l_tree(pT[i + 1])
                        k.attn_l_fin(lsum, t1[i], b, i)
                        if i + 2 < NQB:
                            qT[i + 2] = k.qproj(xhT, i + 2)
                        if not skip_av:
                            k.attn_tail(y, oT, b, i)
                        if i < NQB - 1:
                            spot2(sts)
                        spot2(trs)
                    xhT = nxt

    nc.compile()
    return nc


def _prep_core_inputs(inputs, h):
    """Fold LN affine + attention scale into per-head weights (float64)."""
    x = np.asarray(inputs["x"], np.float32)
    ln_w = np.asarray(inputs["ln_w"], np.float64)
    ln_b = np.asarray(inputs["ln_b"], np.float64)
    sl = slice(h * D, (h + 1) * D)
    scale = float(D) ** -0.5
    q_w = np.asarray(inputs["q_w"], np.float64)[:, sl]
    k_w = np.asarray(inputs["k_w"], np.float64)[:, sl]
    v_w = np.asarray(inputs["v_w"], np.float64)[:, sl]
    o_w = np.asarray(inputs["o_w"], np.float64)[sl, :]
    q_b = np.asarray(inputs["q_b"], np.float64)[sl]
    k_b = np.asarray(inputs["k_b"], np.float64)[sl]
    qw = (ln_w[:, None] * q_w) * scale
    kw = ln_w[:, None] * k_w
    vw = ln_w[:, None] * v_w
    qb = (ln_b @ q_w + q_b) * scale
    kb = ln_b @ k_w + k_b
    return {
        "x": x,
        "qw": qw.astype(np.float32), "kw": kw.astype(np.float32),
        "vw": vw.astype(np.float32), "ow": o_w.astype(np.float32),
        "qb": qb.astype(np.float32), "kb": kb.astype(np.float32),
    }


def kernel(**inputs):
    if "nc" not in _CACHE:
        _CACHE["nc"] = build()
    nc = _CACHE["nc"]

    in_maps = [_prep_core_inputs(inputs, h) for h in range(N_CORES)]
    res = run_bass_kernel_spmd(nc, in_maps, core_ids=list(range(N_CORES)))

    out = np.zeros((B, S, D), np.float64)
    for h in range(N_CORES):
        yh = res.results[h]["y"].astype(np.float64)
        lh = res.results[h]["lsum"].astype(np.float64)
        out += yh / lh[..., None]

    # host-folded constant row: sum_h vb_h @ ow_h + o_b
    ln_b = np.asarray(inputs["ln_b"], np.float64)
    v_w = np.asarray(inputs["v_w"], np.float64)
    v_b = np.asarray(inputs["v_b"], np.float64)
    o_w = np.asarray(inputs["o_w"], np.float64)
    o_b = np.asarray(inputs["o_b"], np.float64)
    vb_full = ln_b @ v_w + v_b            # [D*H]
    out += vb_full @ o_w + o_b
    return out.astype(np.float32)
